# revision 38
# baseline (speedup 1.0000x reference)
"""AUGRU Trainium2 kernel — v8: seq-len-sorted shrinking widths.

The per-core recurrence is latency-bound: each step's serial cycle is
MM -> ACT(sigmoid) -> TT -> MM -> ACT -> TT across engines, ~1586 ns of
fixed per-hop latency plus ~2.7 ns per live batch column (ACT 0.83,
DVE 0.52 x2 each).  Two optimizations over the v5 baseline:

1. Shrinking widths: sequence_length is a host-visible input and
   outputs past it are masked to zero, so columns are sorted by
   seq_len descending and dealt round-robin across 8 cores x NCH
   chains (every chain sees the same width profile).  At step t only
   W_t = ceil(alive(t)/(8*NCH)) columns are still live and every
   per-step instruction is sliced to W_t.  Dead columns hold stale
   values, masked on the host (np.where, since unwritten OUT DRAM can
   be NaN).  The program is compiled per seq-len schedule (cached).

2. Ramp/tail trims: one leading DMA carries biases + weights + step-0
   x/q (bitcast-packed), so the first gate matmul isn't gated on the
   block-0 bulk DMA; the final block's OUT DMA is split so all but the
   last two positions flush while the last steps still run.

NCH=3 (narrower chains) was evaluated and rejected: the in-order ACT
queue cannot follow the 3-way stagger (program order G0,s0,G1,s1,G2,s2
vs time order G0,s2',G1,s0,G2,s1), costing more than the width gain.
"""

import numpy as np

B, T, D, H = 2048, 200, 64, 64
NCORES = 8
NCH = 2                   # chains per core
CW = 128                  # columns per chain
BPAD = NCORES * NCH * CW  # padded batch
KBLK = 16

_CACHE = {}
SCR3_WIN = 44             # steps before the end running the ACT WAR-frontier hack
SLOTS = ((0, 2), (1, 3), (4, 6))  # NCH=2 block slots: x, q, OUT per chain
T0_ORDER = "g1first"      # t=0 emission order variant
EPI_REV = False           # final epilogue chain order
EPI_SPLIT = True          # final epilogue DMAs via SP+ACT queues
HQB = 2                   # hq rotation depth
TPB = 6                   # tmp pool depth
RPB = 8                   # ru pool depth


def _emit_out(nc, OUT, hq, c, p0, p1, eng=None):
    """DMA OUT positions [p0, p1) for chain c.  h_p lives in
    hq[c][((p+1)//KBLK) % 2] slot (p+1) % KBLK; contiguous runs within a
    block share one DMA."""
    eng = eng or nc.sync
    p = p0
    while p < p1:
        b = (p + 1) // KBLK
        s0 = (p + 1) - b * KBLK
        pe = min(p1 - 1, b * KBLK + KBLK - 2)
        n = pe - p + 1
        eng.dma_start(out=OUT[c, :, p:p + n, :],
                      in_=hq[c][b % HQB][0:H, s0:s0 + n, :])
        p += n


def _build(ws):
    """ws: tuple of per-step live widths (len = number of steps)."""
    key = ("nc", NCH, CW, SCR3_WIN, T0_ORDER, EPI_REV, EPI_SPLIT, HQB, TPB, RPB, SLOTS, ws)
    if key in _CACHE:
        return _CACHE[key]

    from contextlib import ExitStack
    import concourse.tile as tile
    from concourse import bacc, mybir

    TS = len(ws)
    CBLK = (TS - 1) // KBLK         # last block with compute steps

    f32 = mybir.dt.float32
    bf16 = mybir.dt.float16
    ALU = mybir.AluOpType
    AF = mybir.ActivationFunctionType

    nc = bacc.Bacc("TRN2", target_bir_lowering=False, debug=False,
                   enable_asserts=False, num_devices=NCORES)

    XD = nc.dram_tensor("x", [NCH, D, T, CW], bf16, kind="ExternalInput").ap()
    QD = nc.dram_tensor("q", [NCH, H, T, CW], bf16, kind="ExternalInput").ap()
    PKW = 3 + 192 + NCH * (CW // 2)  # f32 cols: bpk | wpk(384 bf16) | boot(CW bf16 per chain)
    PK = nc.dram_tensor("pk", [2 * H, PKW], f32, kind="ExternalInput").ap()
    OUT = nc.dram_tensor("out", [NCH, H, T, CW], bf16, kind="ExternalOutput").ap()

    with tile.TileContext(nc) as tc:
        with ExitStack() as ctx:
            consts = ctx.enter_context(tc.tile_pool(name="consts", bufs=1))
            state = ctx.enter_context(tc.tile_pool(name="state", bufs=1))
            gbufs = 150 if NCH == 3 else (158 if KBLK == 16 else 165)
            gpoolG = ctx.enter_context(tc.tile_pool(name="gatesG", bufs=gbufs))
            gpoolS = ctx.enter_context(tc.tile_pool(name="gatesS", bufs=gbufs))
            tpool = ctx.enter_context(tc.tile_pool(name="tmp", bufs=TPB))
            rpool = ctx.enter_context(tc.tile_pool(name="rh", bufs=RPB))
            psbufs = 1 if NCH == 3 else 2
            ps_zg = ctx.enter_context(tc.tile_pool(name="zg", bufs=psbufs, space="PSUM"))
            ps_zc = ctx.enter_context(tc.tile_pool(name="zc", bufs=psbufs, space="PSUM"))

            # ---- shared constants: one DMA carries biases, weights and the
            # steps-0/1 x/q boot data so the first gate matmul starts early ----
            pk_sb = consts.tile([2 * H, PKW], f32, tag="pk")
            nc.sync.dma_start(out=pk_sb[:], in_=PK[:])
            bpk_sb = pk_sb[:, 0:3]
            wpk_sb = pk_sb[0:D, 3:195].bitcast(bf16)          # [64, 384]
            hcw = CW // 2
            boot = [pk_sb[:, 195 + hcw * c:195 + hcw * (c + 1)].bitcast(bf16)
                    for c in range(NCH)]                       # [128, CW] each: step-0 x|q

            # ---- per-chain staged/rotating tiles ----
            xst = [[state.tile([D, KBLK, CW], bf16, tag=f"xst{c}_{j}", name=f"xst{c}_{j}")
                    for j in range(2)] for c in range(NCH)]
            hq = [[state.tile([2 * H, KBLK, CW], bf16, tag=f"hq{c}_{j}", name=f"hq{c}_{j}")
                    for j in range(HQB)] for c in range(NCH)]

            w1x_sb = wpk_sb[:, 0:2 * H]
            w1h_sb = wpk_sb[:, 2 * H:4 * H]
            w2x_sb = wpk_sb[:, 4 * H:5 * H]
            w2h_sb = wpk_sb[:, 5 * H:6 * H]
            bg_sb = bpk_sb[:, 0:1]
            bc_sb = bpk_sb[0:H, 1:2]
            # dummy sigmoid: hoists the ACT table load off the first chain step
            scr = consts.tile([1, 2], bf16, tag="scr")
            scr3 = consts.tile([1, 2], bf16, tag="scr3")
            nc.scalar.activation(scr[:], bpk_sb[0:1, 0:2], AF.Sigmoid)

            ones_hi = [consts.tile([2 * H, CW], bf16, tag=f"oneshi{c}", name=f"oneshi{c}") for c in range(NCH)]
            half_bt = [consts.tile([H, CW], bf16, tag=f"halfbt{c}", name=f"halfbt{c}") for c in range(NCH)]
            half_m = [consts.tile([H, CW], bf16, tag=f"halfm{c}", name=f"halfm{c}") for c in range(NCH)]
            for c in range(NCH):
                nc.gpsimd.memset(ones_hi[c][H:, :], 1.0)
                nc.vector.memset(half_bt[c][:], 0.5)
                nc.vector.memset(half_m[c][:], 0.5)

            hst = [state.tile([2 * H, 2, CW], bf16, tag=f"hst{c}", name=f"hst{c}")
                   for c in range(NCH)]
            btt = [[state.tile([H, CW], bf16, tag=f"btt{c}_{j}", name=f"btt{c}_{j}")
                    for j in range(2)] for c in range(NCH)]
            mtt = [[state.tile([H, CW], bf16, tag=f"mtt{c}_{j}", name=f"mtt{c}_{j}")
                    for j in range(4)] for c in range(NCH)]

            nc.gpsimd.dma_start(out=xst[0][0][:, :, :], in_=XD[0, :, 0:KBLK, :])
            for c in range(1, NCH):
                nc.gpsimd.dma_start(out=xst[c][0][:, :, :], in_=XD[c, :, 0:KBLK, :])
            for c in range(NCH):
                nc.sync.dma_start(out=hq[c][0][H:, 1:KBLK, :], in_=QD[c, :, 1:KBLK, :])

            # DMA/OUT slot schedule within each 8-step block, per chain
            if NCH == 3:
                x_slot = {0: 0, 1: 1, 2: 2}                      # prefetch x blk+1
                q_slot = {0: 3, 1: 4, 2: 5}
                o_slot = {0: 6, 1: 7, 2: 5}
            else:
                x_slot = dict(enumerate(SLOTS[0]))
                q_slot = dict(enumerate(SLOTS[1]))
                o_slot = dict(enumerate(SLOTS[2]))
            bt_engine = nc.vector

            last_out_j = [-1 for _ in range(NCH)]   # last OUT block written per chain
            pos_written = [0 for _ in range(NCH)]   # OUT positions flushed so far

            carry = {}                              # per-chain (zc, a2, bt_cur) refs

            def g_part(t, c):
                W = ws[t]
                blk, ci = divmod(t, KBLK)
                pb = blk % 2
                # spread the block DMAs across the block's steps
                if ci == x_slot[c] and blk + 1 <= CBLK:
                    nb = min((blk + 2) * KBLK, T) - (blk + 1) * KBLK
                    nc.sync.dma_start(out=xst[c][(blk + 1) % 2][:, 0:nb, :],
                                      in_=XD[c, :, (blk + 1) * KBLK:(blk + 1) * KBLK + nb, :])
                if ci == q_slot[c] and blk + 1 <= CBLK:
                    nb = min((blk + 2) * KBLK, T) - (blk + 1) * KBLK
                    nc.sync.dma_start(out=hq[c][(blk + 1) % HQB][H:, 0:nb, :],
                                      in_=QD[c, :, (blk + 1) * KBLK:(blk + 1) * KBLK + nb, :])
                if ci == 6 and blk + 1 <= CBLK:
                    scr2 = tpool.tile([1, 2], bf16, tag=f"scr2{c}", name=f"scr2{c}")
                    nc.vector.tensor_copy(scr2[:], hq[c][(blk + 1) % HQB][H:H + 1, 0, 0:2])
                if ci == o_slot[c] and blk >= 1 and pos_written[c] < blk * KBLK:
                    j = blk - 1
                    nc.sync.dma_start(
                        out=OUT[c, :, j * KBLK:(j + 1) * KBLK - 1, :],
                        in_=hq[c][j % HQB][0:H, 1:KBLK, :])
                    nc.sync.dma_start(
                        out=OUT[c, :, (j + 1) * KBLK - 1:(j + 1) * KBLK, :],
                        in_=hq[c][(j + 1) % HQB][0:H, 0:1, :])
                    last_out_j[c] = j
                    pos_written[c] = (j + 1) * KBLK

                if t == 0:
                    h_tilde_prev = ones_hi[c][H:, 0:W]
                    bt_prev = half_bt[c][:, 0:W]
                    m_prev = half_m[c][:, 0:W]
                else:
                    h_tilde_prev = hst[c][H:, (t - 1) % 2, 0:W]
                    bt_prev = btt[c][(t - 1) % 2][:, 0:W]
                    m_prev = mtt[c][(t - 1) % 4][:, 0:W]

                # gate preactivation (step 0 reads x from the boot cols)
                if t == 0:
                    rhs_x = boot[c][0:D, 0:W]
                else:
                    rhs_x = xst[c][pb][:, ci, 0:W]
                zg = ps_zg.tile([2 * H, CW], f32, tag=f"zg{c}", name=f"zg{c}")
                nc.tensor.matmul(zg[:, 0:W], lhsT=w1x_sb,
                                 rhs=rhs_x,
                                 start=True, stop=(t == 0))
                if t > 0:
                    nc.tensor.matmul(zg[:, 0:W], lhsT=w1h_sb, rhs=bt_prev,
                                     start=False, stop=False)
                    nc.tensor.matmul(zg[:, 0:W], lhsT=w1h_sb, rhs=m_prev,
                                     start=False, stop=True)

                G = gpoolG.tile([2 * H, CW], bf16, tag=f"G{c}", name=f"G{c}")
                nc.scalar.activation(G[:, 0:W], zg[:, 0:W], AF.Sigmoid,
                                     bias=bg_sb if t > 0 else bpk_sb[:, 2:3])

                # DVE: rh (chain), uh, a2, bt
                # t=0: in0 = boot (h-half holds x_0 -> garbage rh, unused since
                # zc skips the rh matmul at t=0; q-half is q_0)
                ru_in0 = boot[c][:, 0:W] if t == 0 else hq[c][blk % HQB][:, ci, 0:W]
                ru = rpool.tile([2 * H, CW], bf16, tag=f"ru{c}", name=f"ru{c}")
                nc.vector.tensor_tensor(ru[:, 0:W], ru_in0,
                                        G[:, 0:W], op=ALU.mult)
                a2 = tpool.tile([H, CW], bf16, tag=f"a2{c}", name=f"a2{c}")
                nc.vector.tensor_scalar(a2[:, 0:W], ru[H:, 0:W], -2.0, 2.0,
                                        op0=ALU.mult, op1=ALU.add)
                bt_cur = btt[c][t % 2]
                nc.vector.tensor_tensor(bt_cur[:, 0:W], ru[H:, 0:W], h_tilde_prev,
                                        op=ALU.mult)

                # candidate
                zc = ps_zc.tile([H, CW], f32, tag=f"zc{c}", name=f"zc{c}")
                nc.tensor.matmul(zc[:, 0:W], lhsT=w2x_sb,
                                 rhs=rhs_x,
                                 start=True, stop=(t == 0))
                if t > 0:
                    nc.tensor.matmul(zc[:, 0:W], lhsT=w2h_sb, rhs=ru[0:H, 0:W],
                                     start=False, stop=True)
                carry[c] = (zc, a2, bt_cur)

            def s_part(t, c):
                W = ws[t]
                zc, a2, bt_cur = carry[c]
                s = gpoolS.tile([H, CW], bf16, tag=f"s{c}", name=f"s{c}")
                nc.scalar.activation(s[:, 0:W], zc[:, 0:W], AF.Sigmoid, bias=bc_sb,
                                     scale=2.0)
                # m (chain, DVE); h~' and h' follow
                m_cur = mtt[c][t % 4]
                nc.vector.tensor_tensor(m_cur[:, 0:W], a2[:, 0:W], s[:, 0:W],
                                        op=ALU.mult)
                nc.vector.tensor_tensor(hst[c][H:, t % 2, 0:W], bt_cur[:, 0:W],
                                        m_cur[:, 0:W], op=ALU.add)
                nblk2, nci = divmod(t + 1, KBLK)
                nc.vector.tensor_scalar(hq[c][nblk2 % HQB][0:H, nci, 0:W],
                                        hst[c][H:, t % 2, 0:W],
                                        -1.0, None, op0=ALU.add)
                if c == NCH - 1 and t >= TS - SCR3_WIN and t % 8 == 5:
                    # advance ACT's DVE-sem frontier: sigmoid WAR conds
                    # on reused G/s tiles get statically subsumed
                    nc.scalar.activation(scr3[:], mtt[NCH - 1][(t - 2) % 4][0:1, 0:2],
                                         AF.Sigmoid)

            if NCH == 3:
                # software-pipelined emission: ACT program order matches the
                # 3-way staggered time order  G0, s2(t-1), G1, s0, G2, s1
                for t in range(TS):
                    g_part(t, 0)
                    if t > 0:
                        s_part(t - 1, 2)
                    if t == TS - 2 and TS - 2 > 0:
                        # early epilogue: positions [pw, TS-2) need only steps
                        # <= TS-3, so their OUT DMA overlaps the last 2 steps
                        for cc in range(NCH):
                            _emit_out(nc, OUT, hq, cc, pos_written[cc], TS - 2)
                            pos_written[cc] = TS - 2
                    g_part(t, 1)
                    s_part(t, 0)
                    g_part(t, 2)
                    s_part(t, 1)
                s_part(TS - 1, 2)
            else:
                for t in range(TS):
                    if t == TS - 2 and TS - 2 > 0:
                        for cc in range(NCH):
                            _emit_out(nc, OUT, hq, cc, pos_written[cc], TS - 2)
                            pos_written[cc] = TS - 2
                    if t == 0 and T0_ORDER != "gsgs":
                        # t=0 anchor order: chain 1 finishes last, so let its
                        # first gate sigmoid ahead of chain 0's candidate
                        parts = {"ggss": [(g_part, 0), (g_part, 1), (s_part, 0), (s_part, 1)],
                                 "ggss2": [(g_part, 0), (g_part, 1), (s_part, 1), (s_part, 0)],
                                 "g1first": [(g_part, 1), (g_part, 0), (s_part, 1), (s_part, 0)],
                                 "g1first2": [(g_part, 1), (g_part, 0), (s_part, 0), (s_part, 1)],
                                 "gsg1": [(g_part, 1), (s_part, 1), (g_part, 0), (s_part, 0)]}[T0_ORDER]
                        for fn_, c_ in parts:
                            fn_(0, c_)
                        continue
                    for c in range(NCH):
                        g_part(t, c)
                        s_part(t, c)

            # Epilogue: flush remaining OUT positions (the last 1-2 steps)
            # from distinct queues so the finals don't serialize on SP
            epi_engs = [nc.sync, nc.scalar]
            for c in (range(NCH) if not EPI_REV else reversed(range(NCH))):
                _emit_out(nc, OUT, hq, c, pos_written[c], TS,
                          eng=epi_engs[c % len(epi_engs)] if EPI_SPLIT else None)

    nc.compile()
    _CACHE[key] = nc
    return nc


def _prep_shared(gate_kernel, gate_bias, cand_kernel, cand_bias):
    bf = np.dtype(np.float16)
    gk = np.asarray(gate_kernel, np.float32)
    gb = np.asarray(gate_bias, np.float32)
    ck = np.asarray(cand_kernel, np.float32)
    cb = np.asarray(cand_bias, np.float32)
    wpk = np.concatenate([gk[:D], gk[D:], ck[:D], ck[D:]], axis=1).astype(bf)  # [64, 384]
    bg = (gb - gk[D:].sum(axis=0)).astype(np.float32)
    bpk = np.zeros((2 * H, 3), np.float32)
    bpk[:, 0] = bg
    bpk[:H, 1] = 2.0 * cb
    bpk[:, 2] = gb
    pk = np.zeros((2 * H, 3 + 192 + NCH * (CW // 2)), np.float32)
    pk[:, 0:3] = bpk
    pk[0:D, 3:195] = np.ascontiguousarray(wpk).view(np.float32)
    return pk


def _schedule(seq_len):
    """Sorted column assignment + per-step width schedule.

    Returns (cols, ws): cols[k, c, i] = padded-batch row for core k,
    chain c, column i (rows >= B are zero padding);  ws[t] = compiled
    live width at step t.
    """
    seq = np.zeros(BPAD, np.int64)
    seq[:B] = np.asarray(seq_len, np.int64)
    order = np.argsort(-seq, kind="stable")          # rank r -> padded row
    cols = np.empty((NCORES, NCH, CW), np.int64)
    for k in range(NCORES):
        for c in range(NCH):
            cols[k, c] = order[k + NCORES * c + NCORES * NCH * np.arange(CW)]
    sdesc = seq[order]
    ts = max(int(sdesc[0]), 1)                       # steps needed = max seq_len
    alive = np.array([(sdesc > t).sum() for t in range(ts)], np.int64)
    ws = np.minimum(CW, np.maximum(2, -(-alive // (NCORES * NCH))))
    return cols, tuple(int(w) for w in ws)


def _run(inputs, trace=False):
    from concourse.bass_utils import run_bass_kernel_spmd

    bf = np.dtype(np.float16)
    rnn_input = np.asarray(inputs["rnn_input"], np.float32)
    seq_len = np.asarray(inputs["sequence_length"], np.int32)
    att = np.asarray(inputs["att_score"], np.float32)
    pk_base = _prep_shared(
        inputs["gate_kernel"], inputs["gate_bias"],
        inputs["cand_kernel"], inputs["cand_bias"])

    cols, ws = _schedule(seq_len)
    nc = _build(ws)

    rnn_pad = np.zeros((BPAD, T, D), np.float32)
    rnn_pad[:B] = rnn_input
    q_pad = np.ones((BPAD, T), np.float32)
    q_pad[:B] = 1.0 - att[:, :, 0]

    in_maps = []
    for k in range(NCORES):
        xi = np.empty((NCH, D, T, CW), bf)
        qi = np.empty((NCH, H, T, CW), bf)
        pk = pk_base.copy()
        for c in range(NCH):
            rows = cols[k, c]
            xi[c] = rnn_pad[rows].transpose(2, 1, 0).astype(bf)
            q = q_pad[rows].T.astype(np.float32)               # [T,CW]
            qi[c] = np.broadcast_to(q[None], (H, T, CW)).astype(bf)
            # boot cols: step 0 x (rows 0:D) and q (rows H:2H)
            bootc = np.zeros((2 * H, CW), bf)
            bootc[0:D] = xi[c][:, 0, :]
            bootc[H:] = qi[c][0, 0, :][None, :]
            hcw = CW // 2
            pk[:, 195 + hcw * c:195 + hcw * (c + 1)] = bootc.view(np.float32)
        in_maps.append({"x": np.ascontiguousarray(xi),
                        "q": np.ascontiguousarray(qi),
                        "pk": pk})

    res = run_bass_kernel_spmd(nc, in_maps, core_ids=list(range(NCORES)), trace=trace)

    out = np.zeros((B, T, H), np.float32)
    t_idx = np.arange(T, dtype=np.int32)
    for k in range(NCORES):
        y = np.asarray(res.results[k]["out"], np.float32)   # [NCH,H,T,CW]
        for c in range(NCH):
            rows = cols[k, c]
            live = rows < B
            r = rows[live]
            hi = y[c].transpose(2, 1, 0)[live]              # [nlive,T,H]
            mask = t_idx[None, :, None] < seq_len[r, None, None]
            out[r] = np.where(mask, hi, 0.0)                # where, not mul: dead
                                                            # positions may be NaN
    return out, res


def kernel(**inputs):
    out, _ = _run(inputs)
    return out


# revision 40
# speedup vs baseline: 1.0057x; 1.0057x over previous
"""AUGRU Trainium2 kernel — v8: seq-len-sorted shrinking widths.

The per-core recurrence is latency-bound: each step's serial cycle is
MM -> ACT(sigmoid) -> TT -> MM -> ACT -> TT across engines, ~1586 ns of
fixed per-hop latency plus ~2.7 ns per live batch column (ACT 0.83,
DVE 0.52 x2 each).  Two optimizations over the v5 baseline:

1. Shrinking widths: sequence_length is a host-visible input and
   outputs past it are masked to zero, so columns are sorted by
   seq_len descending and dealt round-robin across 8 cores x NCH
   chains (every chain sees the same width profile).  At step t only
   W_t = ceil(alive(t)/(8*NCH)) columns are still live and every
   per-step instruction is sliced to W_t.  Dead columns hold stale
   values, masked on the host (np.where, since unwritten OUT DRAM can
   be NaN).  The program is compiled per seq-len schedule (cached).

2. Ramp/tail trims: one leading DMA carries biases + weights + step-0
   x/q (bitcast-packed), so the first gate matmul isn't gated on the
   block-0 bulk DMA; the final block's OUT DMA is split so all but the
   last two positions flush while the last steps still run.

NCH=3 (narrower chains) was evaluated and rejected: the in-order ACT
queue cannot follow the 3-way stagger (program order G0,s0,G1,s1,G2,s2
vs time order G0,s2',G1,s0,G2,s1), costing more than the width gain.
"""

import numpy as np

B, T, D, H = 2048, 200, 64, 64
NCORES = 8
NCH = 2                   # chains per core
CW = 128                  # columns per chain
BPAD = NCORES * NCH * CW  # padded batch
KBLK = 8

_CACHE = {}
SCR3_WIN = 44             # steps before the end running the ACT WAR-frontier hack
SLOTS = ((0, 4), (1, 5), (2, 6))  # NCH=2 block slots: x, q, OUT per chain
T0_ORDER = "g1first"      # t=0 emission order variant
EPI_REV = False           # final epilogue chain order
EPI_SPLIT = True          # final epilogue DMAs via SP+ACT queues
HQB = 2                   # hq rotation depth
TPB = 6                   # tmp pool depth
RPB = 8                   # ru pool depth


def _emit_out(nc, OUT, hq, c, p0, p1, eng=None):
    """DMA OUT positions [p0, p1) for chain c.  h_p lives in
    hq[c][((p+1)//KBLK) % 2] slot (p+1) % KBLK; contiguous runs within a
    block share one DMA."""
    eng = eng or nc.sync
    p = p0
    while p < p1:
        b = (p + 1) // KBLK
        s0 = (p + 1) - b * KBLK
        pe = min(p1 - 1, b * KBLK + KBLK - 2)
        n = pe - p + 1
        eng.dma_start(out=OUT[c, :, p:p + n, :],
                      in_=hq[c][b % HQB][0:H, s0:s0 + n, :])
        p += n


def _build(ws):
    """ws: tuple of per-step live widths (len = number of steps)."""
    key = ("nc", NCH, CW, SCR3_WIN, T0_ORDER, EPI_REV, EPI_SPLIT, HQB, TPB, RPB, SLOTS, ws)
    if key in _CACHE:
        return _CACHE[key]

    from contextlib import ExitStack
    import concourse.tile as tile
    from concourse import bacc, mybir

    TS = len(ws)
    CBLK = (TS - 1) // KBLK         # last block with compute steps

    f32 = mybir.dt.float32
    bf16 = mybir.dt.float16
    ALU = mybir.AluOpType
    AF = mybir.ActivationFunctionType

    nc = bacc.Bacc("TRN2", target_bir_lowering=False, debug=False,
                   enable_asserts=False, num_devices=NCORES)

    XD = nc.dram_tensor("x", [NCH, D, T, CW], bf16, kind="ExternalInput").ap()
    QD = nc.dram_tensor("q", [NCH, H, T, CW], bf16, kind="ExternalInput").ap()
    PKW = 3 + 192 + NCH * (CW // 2)  # f32 cols: bpk | wpk(384 bf16) | boot(CW bf16 per chain)
    PK = nc.dram_tensor("pk", [2 * H, PKW], f32, kind="ExternalInput").ap()
    OUT = nc.dram_tensor("out", [NCH, H, T, CW], bf16, kind="ExternalOutput").ap()

    with tile.TileContext(nc) as tc:
        with ExitStack() as ctx:
            consts = ctx.enter_context(tc.tile_pool(name="consts", bufs=1))
            state = ctx.enter_context(tc.tile_pool(name="state", bufs=1))
            gbufs = 150 if NCH == 3 else 165
            gpoolG = ctx.enter_context(tc.tile_pool(name="gatesG", bufs=gbufs))
            gpoolS = ctx.enter_context(tc.tile_pool(name="gatesS", bufs=gbufs))
            tpool = ctx.enter_context(tc.tile_pool(name="tmp", bufs=TPB))
            rpool = ctx.enter_context(tc.tile_pool(name="rh", bufs=RPB))
            psbufs = 1 if NCH == 3 else 2
            ps_zg = ctx.enter_context(tc.tile_pool(name="zg", bufs=psbufs, space="PSUM"))
            ps_zc = ctx.enter_context(tc.tile_pool(name="zc", bufs=psbufs, space="PSUM"))

            # ---- shared constants: one DMA carries biases, weights and the
            # steps-0/1 x/q boot data so the first gate matmul starts early ----
            pk_sb = consts.tile([2 * H, PKW], f32, tag="pk")
            nc.sync.dma_start(out=pk_sb[:], in_=PK[:])
            bpk_sb = pk_sb[:, 0:3]
            wpk_sb = pk_sb[0:D, 3:195].bitcast(bf16)          # [64, 384]
            hcw = CW // 2
            boot = [pk_sb[:, 195 + hcw * c:195 + hcw * (c + 1)].bitcast(bf16)
                    for c in range(NCH)]                       # [128, CW] each: step-0 x|q

            # ---- per-chain staged/rotating tiles ----
            xst = [[state.tile([D, KBLK, CW], bf16, tag=f"xst{c}_{j}", name=f"xst{c}_{j}")
                    for j in range(2)] for c in range(NCH)]
            hq = [[state.tile([2 * H, KBLK, CW], bf16, tag=f"hq{c}_{j}", name=f"hq{c}_{j}")
                    for j in range(HQB)] for c in range(NCH)]

            w1x_sb = wpk_sb[:, 0:2 * H]
            w1h_sb = wpk_sb[:, 2 * H:4 * H]
            w2x_sb = wpk_sb[:, 4 * H:5 * H]
            w2h_sb = wpk_sb[:, 5 * H:6 * H]
            bg_sb = bpk_sb[:, 0:1]
            bc_sb = bpk_sb[0:H, 1:2]
            # dummy sigmoid: hoists the ACT table load off the first chain step
            scr = consts.tile([1, 2], bf16, tag="scr")
            scr3 = consts.tile([1, 2], bf16, tag="scr3")
            nc.scalar.activation(scr[:], bpk_sb[0:1, 0:2], AF.Sigmoid)

            ones_hi = [consts.tile([2 * H, CW], bf16, tag=f"oneshi{c}", name=f"oneshi{c}") for c in range(NCH)]
            half_bt = [consts.tile([H, CW], bf16, tag=f"halfbt{c}", name=f"halfbt{c}") for c in range(NCH)]
            half_m = [consts.tile([H, CW], bf16, tag=f"halfm{c}", name=f"halfm{c}") for c in range(NCH)]
            for c in range(NCH):
                nc.gpsimd.memset(ones_hi[c][H:, :], 1.0)
                nc.vector.memset(half_bt[c][:], 0.5)
                nc.vector.memset(half_m[c][:], 0.5)

            hst = [state.tile([2 * H, 2, CW], bf16, tag=f"hst{c}", name=f"hst{c}")
                   for c in range(NCH)]
            btt = [[state.tile([H, CW], bf16, tag=f"btt{c}_{j}", name=f"btt{c}_{j}")
                    for j in range(2)] for c in range(NCH)]
            mtt = [[state.tile([H, CW], bf16, tag=f"mtt{c}_{j}", name=f"mtt{c}_{j}")
                    for j in range(4)] for c in range(NCH)]

            nc.gpsimd.dma_start(out=xst[0][0][:, :, :], in_=XD[0, :, 0:KBLK, :])
            for c in range(1, NCH):
                nc.gpsimd.dma_start(out=xst[c][0][:, :, :], in_=XD[c, :, 0:KBLK, :])
            for c in range(NCH):
                nc.sync.dma_start(out=hq[c][0][H:, 1:KBLK, :], in_=QD[c, :, 1:KBLK, :])

            # DMA/OUT slot schedule within each 8-step block, per chain
            if NCH == 3:
                x_slot = {0: 0, 1: 1, 2: 2}                      # prefetch x blk+1
                q_slot = {0: 3, 1: 4, 2: 5}
                o_slot = {0: 6, 1: 7, 2: 5}
            else:
                x_slot = dict(enumerate(SLOTS[0]))
                q_slot = dict(enumerate(SLOTS[1]))
                o_slot = dict(enumerate(SLOTS[2]))
            bt_engine = nc.vector

            last_out_j = [-1 for _ in range(NCH)]   # last OUT block written per chain
            pos_written = [0 for _ in range(NCH)]   # OUT positions flushed so far

            carry = {}                              # per-chain (zc, a2, bt_cur) refs

            def g_part(t, c):
                W = ws[t]
                blk, ci = divmod(t, KBLK)
                pb = blk % 2
                # spread the block DMAs across the block's steps
                if ci == x_slot[c] and blk + 1 <= CBLK:
                    nc.sync.dma_start(out=xst[c][(blk + 1) % 2][:, :, :],
                                      in_=XD[c, :, (blk + 1) * KBLK:(blk + 2) * KBLK, :])
                if ci == q_slot[c] and blk + 1 <= CBLK:
                    nc.sync.dma_start(out=hq[c][(blk + 1) % HQB][H:, :, :],
                                      in_=QD[c, :, (blk + 1) * KBLK:(blk + 2) * KBLK, :])
                if ci == 6 and blk + 1 <= CBLK:
                    scr2 = tpool.tile([1, 2], bf16, tag=f"scr2{c}", name=f"scr2{c}")
                    nc.vector.tensor_copy(scr2[:], hq[c][(blk + 1) % HQB][H:H + 1, 0, 0:2])
                if ci == o_slot[c] and blk >= 1 and pos_written[c] < blk * KBLK:
                    j = blk - 1
                    nc.sync.dma_start(
                        out=OUT[c, :, j * KBLK:(j + 1) * KBLK - 1, :],
                        in_=hq[c][j % HQB][0:H, 1:KBLK, :])
                    nc.sync.dma_start(
                        out=OUT[c, :, (j + 1) * KBLK - 1:(j + 1) * KBLK, :],
                        in_=hq[c][(j + 1) % HQB][0:H, 0:1, :])
                    last_out_j[c] = j
                    pos_written[c] = (j + 1) * KBLK

                if t == 0:
                    h_tilde_prev = ones_hi[c][H:, 0:W]
                    bt_prev = half_bt[c][:, 0:W]
                    m_prev = half_m[c][:, 0:W]
                else:
                    h_tilde_prev = hst[c][H:, (t - 1) % 2, 0:W]
                    bt_prev = btt[c][(t - 1) % 2][:, 0:W]
                    m_prev = mtt[c][(t - 1) % 4][:, 0:W]

                # gate preactivation (step 0 reads x from the boot cols)
                if t == 0:
                    rhs_x = boot[c][0:D, 0:W]
                else:
                    rhs_x = xst[c][pb][:, ci, 0:W]
                zg = ps_zg.tile([2 * H, CW], f32, tag=f"zg{c}", name=f"zg{c}")
                nc.tensor.matmul(zg[:, 0:W], lhsT=w1x_sb,
                                 rhs=rhs_x,
                                 start=True, stop=(t == 0))
                if t > 0:
                    nc.tensor.matmul(zg[:, 0:W], lhsT=w1h_sb, rhs=bt_prev,
                                     start=False, stop=False)
                    nc.tensor.matmul(zg[:, 0:W], lhsT=w1h_sb, rhs=m_prev,
                                     start=False, stop=True)

                G = gpoolG.tile([2 * H, CW], bf16, tag=f"G{c}", name=f"G{c}")
                nc.scalar.activation(G[:, 0:W], zg[:, 0:W], AF.Sigmoid,
                                     bias=bg_sb if t > 0 else bpk_sb[:, 2:3])

                # DVE: rh (chain), uh, a2, bt
                # t=0: in0 = boot (h-half holds x_0 -> garbage rh, unused since
                # zc skips the rh matmul at t=0; q-half is q_0)
                ru_in0 = boot[c][:, 0:W] if t == 0 else hq[c][blk % HQB][:, ci, 0:W]
                ru = rpool.tile([2 * H, CW], bf16, tag=f"ru{c}", name=f"ru{c}")
                nc.vector.tensor_tensor(ru[:, 0:W], ru_in0,
                                        G[:, 0:W], op=ALU.mult)
                a2 = tpool.tile([H, CW], bf16, tag=f"a2{c}", name=f"a2{c}")
                nc.vector.tensor_scalar(a2[:, 0:W], ru[H:, 0:W], -2.0, 2.0,
                                        op0=ALU.mult, op1=ALU.add)
                bt_cur = btt[c][t % 2]
                nc.vector.tensor_tensor(bt_cur[:, 0:W], ru[H:, 0:W], h_tilde_prev,
                                        op=ALU.mult)

                # candidate
                zc = ps_zc.tile([H, CW], f32, tag=f"zc{c}", name=f"zc{c}")
                nc.tensor.matmul(zc[:, 0:W], lhsT=w2x_sb,
                                 rhs=rhs_x,
                                 start=True, stop=(t == 0))
                if t > 0:
                    nc.tensor.matmul(zc[:, 0:W], lhsT=w2h_sb, rhs=ru[0:H, 0:W],
                                     start=False, stop=True)
                carry[c] = (zc, a2, bt_cur)

            def s_part(t, c):
                W = ws[t]
                zc, a2, bt_cur = carry[c]
                s = gpoolS.tile([H, CW], bf16, tag=f"s{c}", name=f"s{c}")
                nc.scalar.activation(s[:, 0:W], zc[:, 0:W], AF.Sigmoid, bias=bc_sb,
                                     scale=2.0)
                # m (chain, DVE); h~' and h' follow
                m_cur = mtt[c][t % 4]
                nc.vector.tensor_tensor(m_cur[:, 0:W], a2[:, 0:W], s[:, 0:W],
                                        op=ALU.mult)
                nc.vector.tensor_tensor(hst[c][H:, t % 2, 0:W], bt_cur[:, 0:W],
                                        m_cur[:, 0:W], op=ALU.add)
                nblk2, nci = divmod(t + 1, KBLK)
                nc.vector.tensor_scalar(hq[c][nblk2 % HQB][0:H, nci, 0:W],
                                        hst[c][H:, t % 2, 0:W],
                                        -1.0, None, op0=ALU.add)
                if c == NCH - 1 and t >= TS - SCR3_WIN and t % 8 == 5:
                    # advance ACT's DVE-sem frontier: sigmoid WAR conds
                    # on reused G/s tiles get statically subsumed
                    nc.scalar.activation(scr3[:], mtt[NCH - 1][(t - 2) % 4][0:1, 0:2],
                                         AF.Sigmoid)

            if NCH == 3:
                # software-pipelined emission: ACT program order matches the
                # 3-way staggered time order  G0, s2(t-1), G1, s0, G2, s1
                for t in range(TS):
                    g_part(t, 0)
                    if t > 0:
                        s_part(t - 1, 2)
                    if t == TS - 2 and TS - 2 > 0:
                        # early epilogue: positions [pw, TS-2) need only steps
                        # <= TS-3, so their OUT DMA overlaps the last 2 steps
                        for cc in range(NCH):
                            _emit_out(nc, OUT, hq, cc, pos_written[cc], TS - 2)
                            pos_written[cc] = TS - 2
                    g_part(t, 1)
                    s_part(t, 0)
                    g_part(t, 2)
                    s_part(t, 1)
                s_part(TS - 1, 2)
            else:
                for t in range(TS):
                    if t == TS - 2 and TS - 2 > 0:
                        for cc in range(NCH):
                            _emit_out(nc, OUT, hq, cc, pos_written[cc], TS - 2)
                            pos_written[cc] = TS - 2
                    if t == 0 and T0_ORDER != "gsgs":
                        # t=0 anchor order: chain 1 finishes last, so let its
                        # first gate sigmoid ahead of chain 0's candidate
                        parts = {"ggss": [(g_part, 0), (g_part, 1), (s_part, 0), (s_part, 1)],
                                 "ggss2": [(g_part, 0), (g_part, 1), (s_part, 1), (s_part, 0)],
                                 "g1first": [(g_part, 1), (g_part, 0), (s_part, 1), (s_part, 0)],
                                 "g1first2": [(g_part, 1), (g_part, 0), (s_part, 0), (s_part, 1)],
                                 "gsg1": [(g_part, 1), (s_part, 1), (g_part, 0), (s_part, 0)]}[T0_ORDER]
                        for fn_, c_ in parts:
                            fn_(0, c_)
                        continue
                    for c in range(NCH):
                        g_part(t, c)
                        s_part(t, c)

            # Epilogue: flush remaining OUT positions (the last 1-2 steps)
            # from distinct queues so the finals don't serialize on SP
            epi_engs = [nc.sync, nc.scalar]
            for c in (range(NCH) if not EPI_REV else reversed(range(NCH))):
                _emit_out(nc, OUT, hq, c, pos_written[c], TS,
                          eng=epi_engs[c % len(epi_engs)] if EPI_SPLIT else None)

    nc.compile()
    _CACHE[key] = nc
    return nc


def _prep_shared(gate_kernel, gate_bias, cand_kernel, cand_bias):
    bf = np.dtype(np.float16)
    gk = np.asarray(gate_kernel, np.float32)
    gb = np.asarray(gate_bias, np.float32)
    ck = np.asarray(cand_kernel, np.float32)
    cb = np.asarray(cand_bias, np.float32)
    wpk = np.concatenate([gk[:D], gk[D:], ck[:D], ck[D:]], axis=1).astype(bf)  # [64, 384]
    bg = (gb - gk[D:].sum(axis=0)).astype(np.float32)
    bpk = np.zeros((2 * H, 3), np.float32)
    bpk[:, 0] = bg
    bpk[:H, 1] = 2.0 * cb
    bpk[:, 2] = gb
    pk = np.zeros((2 * H, 3 + 192 + NCH * (CW // 2)), np.float32)
    pk[:, 0:3] = bpk
    pk[0:D, 3:195] = np.ascontiguousarray(wpk).view(np.float32)
    return pk


def _schedule(seq_len):
    """Sorted column assignment + per-step width schedule.

    Returns (cols, ws): cols[k, c, i] = padded-batch row for core k,
    chain c, column i (rows >= B are zero padding);  ws[t] = compiled
    live width at step t.
    """
    seq = np.zeros(BPAD, np.int64)
    seq[:B] = np.asarray(seq_len, np.int64)
    order = np.argsort(-seq, kind="stable")          # rank r -> padded row
    cols = np.empty((NCORES, NCH, CW), np.int64)
    for k in range(NCORES):
        for c in range(NCH):
            cols[k, c] = order[k + NCORES * c + NCORES * NCH * np.arange(CW)]
    sdesc = seq[order]
    ts = max(int(sdesc[0]), 1)                       # steps needed = max seq_len
    alive = np.array([(sdesc > t).sum() for t in range(ts)], np.int64)
    ws = np.minimum(CW, np.maximum(2, -(-alive // (NCORES * NCH))))
    return cols, tuple(int(w) for w in ws)


def _run(inputs, trace=False):
    from concourse.bass_utils import run_bass_kernel_spmd

    bf = np.dtype(np.float16)
    rnn_input = np.asarray(inputs["rnn_input"], np.float32)
    seq_len = np.asarray(inputs["sequence_length"], np.int32)
    att = np.asarray(inputs["att_score"], np.float32)
    pk_base = _prep_shared(
        inputs["gate_kernel"], inputs["gate_bias"],
        inputs["cand_kernel"], inputs["cand_bias"])

    cols, ws = _schedule(seq_len)
    nc = _build(ws)

    rnn_pad = np.zeros((BPAD, T, D), np.float32)
    rnn_pad[:B] = rnn_input
    q_pad = np.ones((BPAD, T), np.float32)
    q_pad[:B] = 1.0 - att[:, :, 0]

    in_maps = []
    for k in range(NCORES):
        xi = np.empty((NCH, D, T, CW), bf)
        qi = np.empty((NCH, H, T, CW), bf)
        pk = pk_base.copy()
        for c in range(NCH):
            rows = cols[k, c]
            xi[c] = rnn_pad[rows].transpose(2, 1, 0).astype(bf)
            q = q_pad[rows].T.astype(np.float32)               # [T,CW]
            qi[c] = np.broadcast_to(q[None], (H, T, CW)).astype(bf)
            # boot cols: step 0 x (rows 0:D) and q (rows H:2H)
            bootc = np.zeros((2 * H, CW), bf)
            bootc[0:D] = xi[c][:, 0, :]
            bootc[H:] = qi[c][0, 0, :][None, :]
            hcw = CW // 2
            pk[:, 195 + hcw * c:195 + hcw * (c + 1)] = bootc.view(np.float32)
        in_maps.append({"x": np.ascontiguousarray(xi),
                        "q": np.ascontiguousarray(qi),
                        "pk": pk})

    res = run_bass_kernel_spmd(nc, in_maps, core_ids=list(range(NCORES)), trace=trace)

    out = np.zeros((B, T, H), np.float32)
    t_idx = np.arange(T, dtype=np.int32)
    for k in range(NCORES):
        y = np.asarray(res.results[k]["out"], np.float32)   # [NCH,H,T,CW]
        for c in range(NCH):
            rows = cols[k, c]
            live = rows < B
            r = rows[live]
            hi = y[c].transpose(2, 1, 0)[live]              # [nlive,T,H]
            mask = t_idx[None, :, None] < seq_len[r, None, None]
            out[r] = np.where(mask, hi, 0.0)                # where, not mul: dead
                                                            # positions may be NaN
    return out, res


def kernel(**inputs):
    out, _ = _run(inputs)
    return out
